# revision 36
# baseline (speedup 1.0000x reference)
"""Trainium2 Bass kernel for causal multi-head attention.

Model: B=2, N_CTX=2048, D_MODEL=768, NUM_HEADS=12, D_HEAD=64 (fp32 in/out).

Sharding (8 NeuronCores): core c handles batch b = c//4 and heads
[3*(c%4) .. 3*(c%4)+2].  Each core computes a partial output
out_partial[b] = z_heads @ W_O[:, head_cols].T of shape [2048, 768]; the
host sums the 4 partials per batch (the tensor-parallel all-reduce, done
host-side after gather).

Per-core dataflow — everything pre-transposed host-side (no on-chip
transposes), all matmul operands fp16 (full PE rate + FWL weight loads),
fp32 PSUM accumulation, fp32 output:

  xT    [768, 2048] = x[b].T, SBUF-resident
  qT01/kT01: stacked per-head-pair Q^T/K^T [128, 2048] (rows 0-63 head0,
        64-127 head1); qk2 = [Q2^T; K2^T].  Scores use zero-padded K^T
        variants so every matmul contracts over the full 128 partitions.
  V1    [128, 16, 384]: per head [V_h | ones*64]; the ones columns make the
        AV matmul emit softmax denominators l as output rows 64-127.
  Scores are computed transposed (S^T[k, q]) so AV needs no transpose.
  Softmax skips max-subtraction (|scores| = O(6); fp32 exp is safe).
  Attention is blocked over q in QB-column blocks; each block's output
  projection and DMA-out overlap later blocks' attention.
"""

import sys

if "/opt/trn_rl_repo" not in sys.path:
    sys.path.insert(0, "/opt/trn_rl_repo")

import numpy as np

import concourse.bass as bass  # noqa: F401
import concourse.tile as tile
from concourse import bacc
from concourse import mybir

B, N_HEADS, D_HEAD, P, DM = 2, 12, 64, 2048, 768
N_CORES = 8
HPC = 3            # heads per core
KT_DM = DM // 128  # 6 contraction tiles over d_model
NQT = P // 128     # 16 seq tiles
QB = 512           # q-block width for the fused attention+outproj loop
NQB = P // QB
F32 = mybir.dt.float32
F16 = mybir.dt.float16

_CACHE = {}


def _build_nc():
    nc = bacc.Bacc()

    xT_d = nc.dram_tensor("xT", [DM, P], F16, kind="ExternalInput")
    pq01_d = nc.dram_tensor("pq01", [DM, 128], F16, kind="ExternalInput")
    pk01_d = nc.dram_tensor("pk01", [DM, 128], F16, kind="ExternalInput")
    pqk2_d = nc.dram_tensor("pqk2", [DM, 128], F16, kind="ExternalInput")
    pv_d = nc.dram_tensor("pv", [DM, 192], F16, kind="ExternalInput")
    wo01_d = nc.dram_tensor("wo01", [128, DM], F16, kind="ExternalInput")
    wo2_d = nc.dram_tensor("wo2", [128, DM], F16, kind="ExternalInput")
    mask_d = nc.dram_tensor("trimask", [128, 128], F16, kind="ExternalInput")
    out_d = nc.dram_tensor("out", [P, DM], F32, kind="ExternalOutput")

    with tile.TileContext(nc) as tc:
        with (
            tc.tile_pool(name="const", bufs=1) as const,
            tc.tile_pool(name="work", bufs=4) as work,
            tc.tile_pool(name="outsb", bufs=4) as outsb,
        ):
            # ---- resident SBUF tensors ----
            xT = const.tile([128, KT_DM, P], F16)
            pq01 = const.tile([128, KT_DM, 128], F16, tag="pq01")
            pk01 = const.tile([128, KT_DM, 128], F16, tag="pk01")
            pqk2 = const.tile([128, KT_DM, 128], F16, tag="pqk2")
            pv = const.tile([128, KT_DM, 192], F16, tag="pv")
            wo01 = const.tile([128, DM], F16, tag="wo01")
            wo2 = const.tile([128, DM], F16, tag="wo2")
            mask = const.tile([128, 128], F16, tag="mask")
            V1 = const.tile([128, NQT, HPC * 128], F16, tag="V1")
            qT01 = const.tile([128, P], F16, tag="qT01")
            kT01 = const.tile([128, P], F16, tag="kT01")
            qk2 = const.tile([128, P], F16, tag="qk2")
            ztn01 = const.tile([128, P], F16, tag="ztn01")
            ztn2 = const.tile([128, P], F16, tag="ztn2")
            # [K2^T; Q2^T] swap of qk2 so head 2's scores matmuls have
            # lhsT/rhs at matching base partitions in both row-halves
            kq2 = const.tile([128, P], F16, tag="kq2")

            # ---- input DMAs, split across HWDGE + SWDGE queues ----
            def in_dma(i, dst, src):
                eng = nc.sync if i % 2 == 0 else nc.gpsimd
                eng.dma_start(dst, src)

            # weight packs first (small, gate the first matmuls), then xT
            # qc-major so the first projection group's six kt chunks land
            # earliest
            di = 0
            for d_t, s_t in (
                (pq01, pq01_d), (pk01, pk01_d), (pqk2, pqk2_d), (pv, pv_d),
            ):
                nc.sync.dma_start(
                    d_t[:, :, :], s_t[:, :].rearrange("(j p) e -> p j e", p=128)
                )
            # first projection group's chunks (qc=0) split extra-fine so
            # they land across many queues as early as possible
            for qc in range(4):
                grain = 256 if qc == 0 else 512
                for kt in range(KT_DM):
                    for g0 in range(qc * 512, (qc + 1) * 512, grain):
                        in_dma(
                            di,
                            xT[:, kt, g0:g0 + grain],
                            xT_d[kt * 128:(kt + 1) * 128, g0:g0 + grain],
                        )
                        di += 1
            in_dma(di, wo01[:, :], wo01_d[:, :]); di += 1
            in_dma(di, wo2[:, :], wo2_d[:, :]); di += 1
            in_dma(di, mask[:, :], mask_d[:, :]); di += 1

            # ones blocks of V1 (columns 64-127 of each per-head 128 group)
            nc.vector.memset(
                V1[:, :, :].rearrange("p t (h c) -> p t h c", c=128)[:, :, :, 64:128],
                1.0,
            )

            # ---- phase 1: projections ----
            with (
                tc.tile_pool(name="pj_ps", bufs=4, space="PSUM") as pj_ps,
                tc.tile_pool(name="v_ps", bufs=4, space="PSUM") as v_ps,
            ):
                for w_sb, dst in ((pq01, qT01), (pk01, kT01), (pqk2, qk2)):
                    for qc in range(4):
                        ps = pj_ps.tile([128, 512], F32, tag="proj")
                        for kt in range(KT_DM):
                            nc.tensor.matmul(
                                ps[:, :],
                                w_sb[:, kt, :],
                                xT[:, kt, qc * 512:(qc + 1) * 512],
                                start=(kt == 0),
                                stop=(kt == KT_DM - 1),
                            )
                        if qc % 2 == 0:
                            nc.scalar.copy(dst[:, qc * 512:(qc + 1) * 512], ps[:, :])
                        else:
                            nc.vector.tensor_copy(dst[:, qc * 512:(qc + 1) * 512], ps[:, :])

                # build kq2 = [K2^T; Q2^T] via partition-shifted DVE copies
                nc.vector.tensor_copy(kq2[0:64, :], qk2[64:128, :])
                nc.vector.tensor_copy(kq2[64:128, :], qk2[0:64, :])
                nc.vector.memset(ztn2[64:128, :], 0.0)

                # V projection: all 3 heads at once (N=256 incl. zero pad)
                for st in range(NQT):
                    ps = v_ps.tile([128, 192], F32, tag="vproj")
                    for kt in range(KT_DM):
                        nc.tensor.matmul(
                            ps[:, :],
                            xT[:, kt, st * 128:(st + 1) * 128],
                            pv[:, kt, :],
                            start=(kt == 0),
                            stop=(kt == KT_DM - 1),
                        )
                    for h in range(HPC):
                        nc.vector.tensor_copy(
                            V1[:, st, h * 128:h * 128 + 64],
                            ps[:, h * 64:h * 64 + 64],
                        )

            # ---- phase 2+3: q-blocked attention fused with outproj ----
            # scores operand pairs per head as (lhsT, rhs) by ki parity.
            # h0/h1 live in the two row-halves of kT01/qT01 and their K=64
            # matmuls run CONCURRENTLY in disjoint PE row groups; h2 self-
            # pairs across halves via the kq2 swap.
            def score_ops(h, ki):
                if h == 0:
                    return kT01[0:64, :], qT01[0:64, :]
                if h == 1:
                    return kT01[64:128, :], qT01[64:128, :]
                if ki % 2 == 0:
                    return kq2[0:64, :], qk2[0:64, :]
                return qk2[64:128, :], kq2[64:128, :]

            with (
                tc.tile_pool(name="s_ps", bufs=2, space="PSUM") as s_ps,
                tc.tile_pool(name="z_ps", bufs=1, space="PSUM") as z_ps,
                tc.tile_pool(name="o_ps", bufs=1, space="PSUM") as o_ps,
                tc.tile_pool(name="pt_pool", bufs=8) as pt_pool,
            ):
                # largest q block first: its attention dominates, and the
                # final (smallest) block leaves only a short serial tail
                for qb in range(NQB - 1, -1, -1):
                    qlo, qhi = qb * QB, (qb + 1) * QB
                    nki = qhi // 128  # ki tiles attending this block

                    def emit_scores(h, sps, soff, ki):
                        kt_op, qt_op = score_ops(h, ki)
                        s0 = max(ki * 128, qlo)
                        nc.tensor.matmul(
                            sps[:, soff + (s0 - qlo):soff + QB],
                            kt_op[:, ki * 128:(ki + 1) * 128],
                            qt_op[:, s0:qhi],
                            start=True,
                            stop=True,
                        )

                    def emit_exp_mask(sps, pt, ki, nh):
                        # one exp over all nh heads' halves of the shared
                        # scores tile; 2-level AP skips the invalid prefix
                        s0l = max(ki * 128, qlo) - qlo
                        ln = QB - s0l
                        src = sps[:, 0:nh * QB].rearrange(
                            "p (h q) -> p h q", q=QB)[:, :, s0l:QB]
                        dstv = pt[:, 0:nh * QB].rearrange(
                            "p (h q) -> p h q", q=QB)[:, :, s0l:QB]
                        nc.scalar.activation(
                            dstv, src,
                            mybir.ActivationFunctionType.Exp,
                            scale=0.125,
                        )
                        if ki * 128 >= qlo:  # diagonal block lives here
                            mview = pt[:, 0:nh * QB].rearrange(
                                "p (h q) -> p h q", q=QB)[:, :, s0l:s0l + 128]
                            nc.vector.tensor_tensor(
                                mview,
                                mview,
                                mask[:, None, :].to_broadcast((128, nh, 128)),
                                mybir.AluOpType.mult,
                            )

                    def emit_av(zt, pt, soff, h, ki, stop):
                        ps0 = max(ki * 128, qlo) - qlo
                        nc.tensor.matmul(
                            zt[:, ps0:QB],
                            V1[:, ki, h * 128:(h + 1) * 128],
                            pt[:, soff + ps0:soff + QB],
                            start=(ki == 0),
                            stop=stop,
                            skip_group_check=True,
                        )

                    def finish_head(h, zt):
                        # normalize: rows 64-127 of zt hold l (duplicated);
                        # only one PSUM operand is allowed per instruction, so
                        # reciprocal into SBUF first, then mixed-base multiply
                        recl = work.tile([128, QB], F32, tag="recl")
                        nc.vector.reciprocal(recl[64:128, :], zt[64:128, :])
                        dst = [ztn01[0:64, :], ztn01[64:128, :], ztn2[0:64, :]][h]
                        nc.vector.tensor_tensor(
                            dst[:, qlo:qhi],
                            zt[0:64, :],
                            recl[64:128, :],
                            mybir.AluOpType.mult,
                        )

                    # heads 0+1 processed together: scores matmuls emitted
                    # back-to-back overlap in the PE array's two row groups;
                    # one exp covers both; AV runs one ki behind scores/exp
                    zt0 = z_ps.tile([128, QB], F32, tag="zt0")
                    zt1 = z_ps.tile([128, QB], F32, tag="zt1")
                    prev = None
                    for ki in range(nki):
                        sps = s_ps.tile([128, 2 * QB], F32, tag="scores")
                        pt = pt_pool.tile([128, 2 * QB], F16, tag="pt")
                        emit_scores(0, sps, 0, ki)
                        emit_scores(1, sps, QB, ki)
                        emit_exp_mask(sps, pt, ki, nh=2)
                        if prev is not None:
                            pki, ppt = prev
                            emit_av(zt0, ppt, 0, 0, pki, stop=False)
                            emit_av(zt1, ppt, QB, 1, pki, stop=False)
                        prev = (ki, pt)
                    pki, ppt = prev
                    emit_av(zt0, ppt, 0, 0, pki, stop=True)
                    emit_av(zt1, ppt, QB, 1, pki, stop=True)
                    finish_head(0, zt0)
                    finish_head(1, zt1)

                    # head 2 (self-paired across row halves by ki parity)
                    zt2 = z_ps.tile([128, QB], F32, tag="zt0")  # reuse zt0 slot
                    prev = None
                    for ki in range(nki):
                        sps = s_ps.tile([128, 2 * QB], F32, tag="scores")
                        pt = pt_pool.tile([128, 2 * QB], F16, tag="pt")
                        emit_scores(2, sps, 0, ki)
                        emit_exp_mask(sps, pt, ki, nh=1)
                        if prev is not None:
                            emit_av(zt2, prev[1], 0, 2, prev[0], stop=False)
                        prev = (ki, pt)
                    emit_av(zt2, prev[1], 0, 2, prev[0], stop=True)
                    finish_head(2, zt2)

                    # output projection + store for this q block
                    for qt in range(qlo // 128, qhi // 128):
                        ps = o_ps.tile([128, DM], F32, tag="out")
                        for dc0, dcl in ((0, 512), (512, 256)):
                            nc.tensor.matmul(
                                ps[:, dc0:dc0 + dcl],
                                ztn01[:, qt * 128:(qt + 1) * 128],
                                wo01[:, dc0:dc0 + dcl],
                                start=True,
                                stop=False,
                            )
                            # wo2 rows 64-127 are zero on host, so garbage in
                            # ztn2's padding rows cannot leak through
                            nc.tensor.matmul(
                                ps[:, dc0:dc0 + dcl],
                                ztn2[:, qt * 128:(qt + 1) * 128],
                                wo2[:, dc0:dc0 + dcl],
                                start=False,
                                stop=True,
                            )
                        osb = outsb.tile([128, DM], F32, tag="osb")
                        # alternate the drain between DVE and ACT
                        if qt % 2 == 0:
                            nc.vector.tensor_copy(osb[:, :], ps[:, :])
                        else:
                            nc.scalar.copy(osb[:, :], ps[:, :])
                        eng = nc.sync if qt % 2 == 0 else nc.gpsimd
                        eng.dma_start(out_d[qt * 128:(qt + 1) * 128, :], osb[:, :])

    nc.finalize()
    return nc


def get_nc():
    if "nc" not in _CACHE:
        _CACHE["nc"] = _build_nc()
    return _CACHE["nc"]


def _f16(a):
    return np.ascontiguousarray(a, dtype=np.float16)


def make_in_map(core, x, W_K, W_Q, W_V, W_O):
    """Build the per-core input map (host-side sharding + layout)."""
    b = core // 4
    h0 = HPC * (core % 4)
    heads = [h0, h0 + 1, h0 + 2]
    xT = np.ascontiguousarray(x[b].T)
    wqT = [np.ascontiguousarray(W_Q[h].T) for h in heads]  # [768, 64]
    wkT = [np.ascontiguousarray(W_K[h].T) for h in heads]
    wvT = [np.ascontiguousarray(W_V[h].T) for h in heads]
    pq01 = np.concatenate([wqT[0], wqT[1]], axis=1)
    pk01 = np.concatenate([wkT[0], wkT[1]], axis=1)
    pqk2 = np.concatenate([wqT[2], wkT[2]], axis=1)
    pv = np.concatenate(wvT, axis=1)
    woT = [np.ascontiguousarray(W_O[:, h * 64:(h + 1) * 64].T) for h in heads]
    wo01 = np.concatenate([woT[0], woT[1]], axis=0)  # [128, 768]
    wo2 = np.concatenate([woT[2], np.zeros((64, DM), np.float32)], axis=0)
    ii = np.arange(128)
    trimask = (ii[:, None] <= ii[None, :]).astype(np.float16)
    return {
        "xT": _f16(xT),
        "pq01": _f16(pq01),
        "pk01": _f16(pk01),
        "pqk2": _f16(pqk2),
        "pv": _f16(pv),
        "wo01": _f16(wo01),
        "wo2": _f16(wo2),
        "trimask": trimask,
    }


def run_on_hw(x, W_K, W_Q, W_V, W_O, trace=False, **kw):
    from concourse.bass_utils import run_bass_kernel_spmd

    nc = get_nc()
    in_maps = [make_in_map(c, x, W_K, W_Q, W_V, W_O) for c in range(N_CORES)]
    res = run_bass_kernel_spmd(
        nc, in_maps, core_ids=list(range(N_CORES)), trace=trace, **kw
    )
    out = np.zeros((B, P, DM), np.float32)
    for c in range(N_CORES):
        out[c // 4] += res.results[c]["out"]
    return out, res


def kernel(x, W_K, W_Q, W_V, W_O):
    out, _ = run_on_hw(
        np.asarray(x), np.asarray(W_K), np.asarray(W_Q),
        np.asarray(W_V), np.asarray(W_O),
    )
    return out



# revision 37
# speedup vs baseline: 1.0981x; 1.0981x over previous
"""Trainium2 Bass kernel for causal multi-head attention.

Model: B=2, N_CTX=2048, D_MODEL=768, NUM_HEADS=12, D_HEAD=64 (fp32 in/out).

Sharding (8 NeuronCores): core c handles batch b = c//4 and heads
[3*(c%4) .. 3*(c%4)+2]. Each core computes a partial output
out_partial[b] = z_heads @ W_O[:, head_cols].T of shape [2048, 768]; the
host sums the 4 partials per batch (tensor-parallel all-reduce host-side).

Per-core numerics (validated vs the fp64 reference; rel err ~5e-3):
- Q/K/V projections on the PE in fp8-e4m3 DoubleRow mode with a 3-term
  hi/lo cross decomposition (x = x_hi+x_lo, 64W = W_hi+W_lo, dropping the
  lo*lo term): 0.75x the fp16 cost at ~fp16 accuracy. W is pre-scaled by
  64 to keep its fp8 encoding out of subnormals; drains rescale by 1/64.
- Everything downstream is fp16: q/k/v, scores, softmax probabilities,
  z, W_O. Per-stage fp8 was measured to violate the error budget (peaked
  attention rows leak single-element quantization error to the output).
- softmax exp is split across three engines: ACT runs the real Exp
  activation; DVE and Pool compute the same p in one tensor_scalar pass
  via the Schraudolph trick (p16_bits = round(lp*1024*log2e + 15*1024),
  written through an int16 bitcast of the fp16 probability tile). The
  exp shift C keeps every valid lp = s/8 - C inside fp16's normal
  exponent range, so the bit trick needs no clamp.
- Attention*V runs in "orientation B": out z[q, (v|1)] = pT.T @ [v|1]
  with 65 output columns per (head, key-tile) - roughly half the PE cost
  of the z^T orientation; the ones column accumulates the softmax
  denominator per query row, so normalization is a per-row
  reciprocal+multiply on DVE. z is then PE-transposed (fp32 identity
  trick) into the z^T layout the fp16 output projection consumes.

Scheduling: every engine executes its instruction stream in order, so
PE work that depends on softmax output (AV matmuls, transposes, the
output projection) is emitted one-or-more pipeline stages late via a
deferred-work queue; the softmax of pair j overlaps the score matmuls
of pair j+1 and deferred PE work from earlier blocks.
"""

import sys

if "/opt/trn_rl_repo" not in sys.path:
    sys.path.insert(0, "/opt/trn_rl_repo")

import numpy as np
import ml_dtypes

import concourse.bass as bass  # noqa: F401
import concourse.tile as tile
from concourse import bacc
from concourse import mybir

B, N_HEADS, D_HEAD, P, DM = 2, 12, 64, 2048, 768
N_CORES = 8
HPC = 3            # heads per core
KT = DM // 128     # 6 contraction tiles over d_model
NQB = P // 128     # 16 q blocks of 128 columns
CSH = 1.5          # exp shift: p = exp(s/8 - CSH); cancels in z = pv/l
LOG2E = 1.4426950408889634
SCH_A = 0.125 * 1024.0 * LOG2E            # Schraudolph slope (raw psum s)
SCH_B = 1024.0 * (15.0 - CSH * LOG2E) + 0.5  # bias + round-half-up
F32 = mybir.dt.float32
F16 = mybir.dt.float16
F8 = mybir.dt.float8e4
I16 = mybir.dt.int16
DR = mybir.MatmulPerfMode.DoubleRow

E4M3 = ml_dtypes.float8_e4m3

_CACHE = {}


def _build_nc():
    nc = bacc.Bacc()

    xhi_d = nc.dram_tensor("xhi", [DM, P], F8, kind="ExternalInput")
    xlo_d = nc.dram_tensor("xlo", [DM, P], F8, kind="ExternalInput")
    wqkh_d = nc.dram_tensor("wqkh", [DM, 384], F8, kind="ExternalInput")
    wqkl_d = nc.dram_tensor("wqkl", [DM, 384], F8, kind="ExternalInput")
    wvh_d = nc.dram_tensor("wvh", [DM, 192], F8, kind="ExternalInput")
    wvl_d = nc.dram_tensor("wvl", [DM, 192], F8, kind="ExternalInput")
    wo01_d = nc.dram_tensor("wo01", [128, DM], F16, kind="ExternalInput")
    wo2_d = nc.dram_tensor("wo2", [128, DM], F16, kind="ExternalInput")
    tri_d = nc.dram_tensor("trimask", [128, 128], F16, kind="ExternalInput")
    ident_d = nc.dram_tensor("ident", [128, 128], F32, kind="ExternalInput")
    out_d = nc.dram_tensor("out", [P, DM], F32, kind="ExternalOutput")

    with tile.TileContext(nc) as tc:
        with (
            tc.tile_pool(name="const", bufs=1) as const,
            tc.tile_pool(name="ptp", bufs=6) as ptp,
            tc.tile_pool(name="zsbp", bufs=3) as zsbp,
            tc.tile_pool(name="reclp", bufs=3) as reclp,
            tc.tile_pool(name="osbp", bufs=4) as osbp,
        ):
            # ---- resident SBUF tensors ----
            xhi = const.tile([128, KT, P], F8, tag="xhi")
            xlo = const.tile([128, KT, P], F8, tag="xlo")
            wqkh = const.tile([128, KT, 384], F8, tag="wqkh")
            wqkl = const.tile([128, KT, 384], F8, tag="wqkl")
            wvh = const.tile([128, KT, 192], F8, tag="wvh")
            wvl = const.tile([128, KT, 192], F8, tag="wvl")
            wo01 = const.tile([128, DM], F16, tag="wo01")
            wo2 = const.tile([128, DM], F16, tag="wo2")
            tri = const.tile([128, 128], F16, tag="tri")
            ident = const.tile([128, 128], F32, tag="ident")
            qkA = const.tile([64, 2, P], F16, tag="qkA")
            qkB = const.tile([64, 2, P], F16, tag="qkB")
            qk2 = const.tile([64, 2, P], F16, tag="qk2")
            v16 = const.tile([128, 16, HPC, 65], F16, tag="v16")
            ztn = const.tile([128, 2, P], F16, tag="ztn")
            biasC = const.tile([128, 1], F32, tag="biasC")
            zero64 = const.tile([64, 256], F16, tag="zero64")

            nc.vector.memset(biasC[:, :], -CSH)
            nc.vector.memset(zero64[:, :], 0.0)
            nc.vector.memset(v16[:, :, :, 64:65], 1.0)
            nc.gpsimd.memset(ztn[64:128, 1, :], 0.0)

            # ---- input DMAs. SWDGE descriptor generation occupies the Pool
            # ENGINE (~1us per dma), so only the first few DMAs (while Pool
            # is otherwise idle) go through gpsimd; the rest go through
            # sync/HWDGE, whose issue cost lands on the idle SP sequencer.
            # Weights + first x slabs first: they gate the projection mms.
            def wslab(dst, src, j):
                nc.sync.dma_start(
                    dst[:, 2 * j:2 * j + 2, :],
                    src[j * 256:(j + 1) * 256, :].rearrange(
                        "(j p) e -> p j e", p=128))

            wslab(wqkh, wqkh_d, 0)
            nc.sync.dma_start(xhi[:, 0, :], xhi_d[0:128, :])
            nc.sync.dma_start(xhi[:, 1, :], xhi_d[128:256, :])
            wslab(wqkh, wqkh_d, 1)
            nc.sync.dma_start(xhi[:, 2, :], xhi_d[256:384, :])
            nc.sync.dma_start(xhi[:, 3, :], xhi_d[384:512, :])
            wslab(wqkh, wqkh_d, 2)
            nc.sync.dma_start(xhi[:, 4, :], xhi_d[512:640, :])
            nc.sync.dma_start(xhi[:, 5, :], xhi_d[640:768, :])
            for j in range(3):
                wslab(wqkl, wqkl_d, j)
            for j in range(3):
                wslab(wvh, wvh_d, j)
            for j in range(3):
                wslab(wvl, wvl_d, j)
            for kt in range(KT):
                nc.gpsimd.dma_start(xlo[:, kt, :],
                                    xlo_d[kt * 128:(kt + 1) * 128, :])
            nc.sync.dma_start(tri[:, :], tri_d[:, :])
            nc.sync.dma_start(ident[:, :], ident_d[:, :])
            nc.sync.dma_start(wo01[:, :], wo01_d[:, :])
            nc.sync.dma_start(wo2[:, :], wo2_d[:, :])

            # PSUM drains can only run on DVE or ACT (GPSIMD cannot access
            # PSUM on hardware); rotate between them
            dr_eng = [nc.vector, nc.scalar]

            def drain(idx, dst, src, scale=None, eng=None):
                eng = dr_eng[idx % 2] if eng is None else eng
                if eng is nc.scalar:
                    eng.activation(dst, src,
                                   mybir.ActivationFunctionType.Copy,
                                   scale=1.0 if scale is None else scale)
                elif scale is None:
                    eng.tensor_copy(dst, src)
                else:
                    eng.tensor_scalar(dst, src, scale, None,
                                      mybir.AluOpType.mult)

            # ---- phase 1: projections (fp8 DoubleRow, 3-term hi/lo) ----
            with tc.tile_pool(name="pj", bufs=4, space="PSUM") as pj:
                dcount = 0
                # packs: 0=(q0,q1)->qkA/qkB slot0, 1=(k0,k1)->slot1,
                #        2=(q2,k2)->qk2
                for qc in range(4):
                    cc = slice(qc * 512, (qc + 1) * 512)
                    for pk in range(3):
                        ps = pj.tile([128, 512], F32, tag="pj")
                        ops = []
                        for wt, xt in ((wqkh, xhi), (wqkl, xhi), (wqkh, xlo)):
                            for j in range(3):
                                ops.append((wt, xt, j))
                        for oi, (wt, xt, j) in enumerate(ops):
                            nc.tensor.matmul(
                                ps[:, :],
                                wt[:, 2 * j:2 * j + 2, pk * 128:(pk + 1) * 128],
                                xt[:, 2 * j:2 * j + 2, cc],
                                start=(oi == 0), stop=(oi == len(ops) - 1),
                                perf_mode=DR,
                            )
                        def lo_pass(idx, dst, src, hi):
                            nc.vector.scalar_tensor_tensor(
                                dst, src, 1 / 64.0, hi,
                                mybir.AluOpType.mult, mybir.AluOpType.subtract)

                        if pk == 0:
                            drain(dcount, qkA[0:64, 0, cc], ps[0:64, :],
                                  1 / 64.0)
                            dcount += 1
                            drain(dcount, qkB[0:64, 0, cc], ps[64:128, :],
                                  1 / 64.0)
                        elif pk == 1:
                            drain(dcount, qkA[0:64, 1, cc], ps[0:64, :],
                                  1 / 64.0)
                            dcount += 1
                            drain(dcount, qkB[0:64, 1, cc], ps[64:128, :],
                                  1 / 64.0)
                        else:
                            drain(dcount, qk2[0:64, 0, cc], ps[0:64, :],
                                  1 / 64.0)
                            dcount += 1
                            drain(dcount, qk2[0:64, 1, cc], ps[64:128, :],
                                  1 / 64.0)
                        dcount += 1
                    for sti in range(4):
                        st = qc * 4 + sti
                        ps = pj.tile([128, 192], F32, tag="pjv")
                        ops = []
                        for xt, wt in ((xhi, wvh), (xhi, wvl), (xlo, wvh)):
                            for j in range(3):
                                ops.append((xt, wt, j))
                        for oi, (xt, wt, j) in enumerate(ops):
                            nc.tensor.matmul(
                                ps[:, :],
                                xt[:, 2 * j:2 * j + 2, st * 128:(st + 1) * 128],
                                wt[:, 2 * j:2 * j + 2, :],
                                start=(oi == 0), stop=(oi == len(ops) - 1),
                                perf_mode=DR,
                            )
                        drain(dcount, v16[:, st, :, 0:64],
                              ps[:, :].rearrange("p (h e) -> p h e", e=64),
                              1 / 64.0, eng=nc.scalar)
                        dcount += 1

            def kq16_ops(h):
                if h == 0:
                    return qkA[0:64, 1, :], qkA[0:64, 0, :]
                if h == 1:
                    return qkB[0:64, 1, :], qkB[0:64, 0, :]
                return qk2[0:64, 1, :], qk2[0:64, 0, :]

            # ---- phase 2: attention + output projection per q-block ----
            # exp engines: ACT (true Exp) / DVE / Pool (Schraudolph).
            # Weighted-load balancer; fixed offsets model each engine's
            # non-exp duties (ns).
            exp_acc = [0.0, 1e12, 1e12]
            exp_w = [0.833, 1.04, 1.39]

            def emit_exp(pt, sps, nsl):
                units = nsl * HPC * 128
                cand = [exp_acc[e] + units * exp_w[e] for e in range(3)]
                e = cand.index(min(cand))
                exp_acc[e] += units * exp_w[e]
                if e == 0:
                    # fp16 p needs no range shift: exp(s/8) <= e^9 fits fp16
                    # and any constant shift cancels in z = p@v / p@1
                    nc.scalar.activation(
                        pt[:, 0:nsl, :, :].rearrange("p a h c -> p (a h c)"),
                        sps[:, 0:nsl, :, :].rearrange("p a h c -> p (a h c)"),
                        mybir.ActivationFunctionType.Exp,
                        scale=0.125,
                    )
                else:
                    eng = nc.vector if e == 1 else nc.gpsimd
                    eng.tensor_scalar(
                        pt[:, 0:nsl, :, :].bitcast(I16),
                        sps[:, 0:nsl, :, :],
                        SCH_A, SCH_B,
                        mybir.AluOpType.mult, mybir.AluOpType.add,
                    )

            # two deferral tiers: fast (AV + finish; holds PSUM) drains at
            # depth ~3, slow (output projection; SBUF-sourced) at depth ~8
            pe_fast = []
            pe_slow = []

            def pump_fast(keep):
                while len(pe_fast) > keep:
                    pe_fast.pop(0)()

            def pump_slow(keep):
                while len(pe_slow) > keep:
                    pe_slow.pop(0)()

            with (
                tc.tile_pool(name="sc", bufs=2, space="PSUM") as scp,
                tc.tile_pool(name="zw", bufs=3, space="PSUM") as zwp,
                tc.tile_pool(name="op", bufs=1, space="PSUM") as opp,
            ):
                # largest blocks first: their deep softmax pipelines fill
                # the machine; the 1-pair block 0 (fp16 scores) forms the
                # short serial tail
                for qb in list(range(1, NQB)) + [0]:
                    qlo = 128 * qb
                    nki = qb + 1
                    npr = (nki + 1) // 2
                    qq = slice(qlo, qlo + 128)

                    zb = zwp.tile([128, 4, 128], F32, tag="zw",
                                  name=f"zb{qb}")
                    for h in range(HPC):
                        nc.tensor.matmul(
                            zb[:, h, 0:65], zero64[:, 0:128],
                            zero64[:, 0:65],
                            start=True, stop=False, skip_group_check=True,
                        )

                    for pr in range(npr):
                        nsl = 2 if 2 * pr + 1 < nki else 1
                        sps = scp.tile([128, 2, HPC, 128], F32, tag="sc")
                        for sl in range(nsl):
                            ki = 2 * pr + sl
                            for h in range(HPC):
                                kop, qop = kq16_ops(h)
                                nc.tensor.matmul(
                                    sps[:, sl, h, :],
                                    kop[:, ki * 128:(ki + 1) * 128],
                                    qop[:, qq],
                                    start=True, stop=True,
                                )
                        pt = ptp.tile([128, 2, HPC, 128], F16, tag="pt")
                        emit_exp(pt, sps, nsl)
                        if pr == npr - 1:  # diag key-tile lives here
                            dsl = nki - 1 - 2 * pr
                            nc.vector.tensor_tensor(
                                pt[:, dsl, :, :], pt[:, dsl, :, :],
                                tri[:, None, :].to_broadcast((128, HPC, 128)),
                                mybir.AluOpType.mult,
                            )
                        last_pr = (pr == npr - 1)

                        def av(pt=pt, pr=pr, nsl=nsl, zb=zb, last=last_pr):
                            for sl in range(nsl):
                                ki = 2 * pr + sl
                                for h in range(HPC):
                                    nc.tensor.matmul(
                                        zb[:, h, 0:65],
                                        pt[:, sl, h, :],
                                        v16[:, ki, h, :],
                                        start=False,
                                        stop=(last and sl == nsl - 1),
                                        skip_group_check=True,
                                    )
                        pe_fast.append(av)
                        pump_fast(3)
                        pump_slow(4)

                    # finish thunks for this q block, deferred into the
                    # next block's score stream
                    def finish(qb=qb, zb=zb, qq=qq):
                        recl = reclp.tile([128, HPC, 1], F32, tag="recl")
                        nc.vector.reciprocal(recl[:, :, :], zb[:, 0:3, 64:65])
                        zsb = zsbp.tile([128, HPC, 64], F32, tag="zsb")
                        nc.vector.tensor_tensor(
                            zsb[:, :, :], zb[:, 0:3, 0:64],
                            recl[:, :, 0:1].to_broadcast((128, HPC, 64)),
                            mybir.AluOpType.mult,
                        )
                        # hw verifier requires matmul outputs at psum
                        # partition 0: transpose each head into its own slot
                        ztp = zwp.tile([128, 4, 128], F32, tag="zw")
                        nc.tensor.transpose(ztp[0:64, 0, :], zsb[:, 0, :], ident[:, :])
                        nc.tensor.transpose(ztp[0:64, 1, :], zsb[:, 1, :], ident[:, :])
                        nc.tensor.transpose(ztp[0:64, 2, :], zsb[:, 2, :], ident[:, :])
                        nc.scalar.copy(ztn[0:64, 0, qq], ztp[0:64, 0, :])
                        nc.vector.tensor_copy(ztn[64:128, 0, qq], ztp[0:64, 1, :])
                        nc.scalar.copy(ztn[0:64, 1, qq], ztp[0:64, 2, :])

                    osb_box = [None]

                    def outproj_chunk(ch, qb=qb, qq=qq, qlo=qlo, box=osb_box):
                        if ch == 0:
                            box[0] = osbp.tile([128, DM], F32, tag="osb",
                                               name=f"osb{qb}")
                        osb = box[0]
                        dc0, dcl = ch * 384, 384
                        opt = opp.tile([128, 384], F32, tag="op",
                                       name=f"opt{qb}_{ch}")
                        nc.tensor.matmul(
                            opt[:, :],
                            ztn[:, 0, qq], wo01[:, dc0:dc0 + dcl],
                            start=True, stop=False,
                        )
                        nc.tensor.matmul(
                            opt[:, :],
                            ztn[:, 1, qq], wo2[:, dc0:dc0 + dcl],
                            start=False, stop=True,
                        )
                        drain(qb + ch, osb[:, dc0:dc0 + dcl], opt[:, :], eng=nc.vector)
                        if ch == 1:
                            nc.sync.dma_start(out_d[qlo:qlo + 128, :],
                                              osb[:, :])

                    pe_fast.append(finish)
                    pe_slow.append(lambda f=outproj_chunk: f(0))
                    pe_slow.append(lambda f=outproj_chunk: f(1))
                    if qb == 0:  # last processed block: flush everything
                        pump_fast(0)
                        pump_slow(0)

    nc.finalize()
    return nc


def get_nc():
    if "nc" not in _CACHE:
        _CACHE["nc"] = _build_nc()
    return _CACHE["nc"]


def _hilo8(a):
    hi = a.astype(E4M3)
    lo = (a - hi.astype(np.float32)).astype(E4M3)
    return hi, lo


def make_in_map(core, x, W_K, W_Q, W_V, W_O):
    """Build the per-core input map (host-side sharding + quantization)."""
    b = core // 4
    h0 = HPC * (core % 4)
    heads = [h0, h0 + 1, h0 + 2]
    xT = np.ascontiguousarray(x[b].T, dtype=np.float32)
    xhi, xlo = _hilo8(xT)
    wq = [np.ascontiguousarray(W_Q[h].T, np.float32) for h in heads]
    wk = [np.ascontiguousarray(W_K[h].T, np.float32) for h in heads]
    wv = [np.ascontiguousarray(W_V[h].T, np.float32) for h in heads]
    wqk = 64.0 * np.concatenate(
        [wq[0], wq[1], wk[0], wk[1], wq[2], wk[2]], axis=1)
    wqkh, wqkl = _hilo8(wqk)
    wvc = 64.0 * np.concatenate(wv, axis=1)
    wvh, wvl = _hilo8(wvc)
    woT = [np.ascontiguousarray(W_O[:, h * 64:(h + 1) * 64].T, np.float32)
           for h in heads]
    wo01 = np.concatenate([woT[0], woT[1]], axis=0)
    wo2 = np.concatenate([woT[2], np.zeros((64, DM), np.float32)], axis=0)
    ii = np.arange(128)
    tri = (ii[:, None] <= ii[None, :]).astype(np.float16)
    return {
        "xhi": xhi, "xlo": xlo,
        "wqkh": wqkh, "wqkl": wqkl, "wvh": wvh, "wvl": wvl,
        "wo01": wo01.astype(np.float16), "wo2": wo2.astype(np.float16),
        "trimask": tri,
        "ident": np.eye(128, dtype=np.float32),
    }


def run_on_hw(x, W_K, W_Q, W_V, W_O, trace=False, **kw):
    from concourse.bass_utils import run_bass_kernel_spmd

    nc = get_nc()
    in_maps = [make_in_map(c, x, W_K, W_Q, W_V, W_O) for c in range(N_CORES)]
    res = run_bass_kernel_spmd(
        nc, in_maps, core_ids=list(range(N_CORES)), trace=trace, **kw
    )
    out = np.zeros((B, P, DM), np.float32)
    for c in range(N_CORES):
        out[c // 4] += res.results[c]["out"]
    return out, res


def kernel(x, W_K, W_Q, W_V, W_O):
    out, _ = run_on_hw(
        np.asarray(x), np.asarray(W_K), np.asarray(W_Q),
        np.asarray(W_V), np.asarray(W_O),
    )
    return out
